# revision 2
# baseline (speedup 1.0000x reference)
"""Trainium2 Bass kernel for nn_Attention_86423331930617 — head-sharded.

Sharding: core c owns attention HEAD c for ALL 16 batches (instead of
batch-sharding).  The hypernetwork W3/V3 big weights stay column-sharded
(W3 by head e-columns, V3 by iv-rows), so every core computes exactly the
hypernet slice it needs locally -> NO collective before attention.  The
single AllToAll moves head-outputs to batch-owners AFTER attention and is
split in two (batch 2d | batch 2d+1 per dest) so the first one overlaps
the second half of compute.

Per batch b (my head h):
  ehq   = exp(h_slice(b))            [128d, 96e]   (96 = q|k|v 32 each)
  r96   = 1/colsum_d(ehq)            [96, 1]       per-partition recip
  qkT   = (x[b]^T_d @ ehq_qk) * r96  [64, 600]     softmax-normalized q,k
  v     = x[b] @ ehq_v               [600, 32]     UNnormalized (r_v ships)
  dots  = k^T q                      5x [128m, 600n] PSUM
  e     = exp(dots)  (2 big ACT insts over multi-bank PSUM)
  em    = e * maskT[b]               (DVE + GpSimd split)
  s     = rowsum_m(e)  (ones-matmul rows 32/65 of the po PSUM block)
  po    = v^T @ em                   [32, 600] unnormalized
  ship [po|s] + r_v + v3h_slice(b) via AllToAll row b//2.
Batch-owner (after a2a): R[e,n] = outer(r_v, 1/s_h) per head (rank-8 PSUM
build), out_norm = outT * R, ow = softmax(v3h) via per-partition recip,
y^T = ehw^T @ out_norm * r_ow.
"""
import sys

sys.path.insert(0, "/opt/trn_rl_repo")
if "/root/.axon_site" not in sys.path:
    sys.path.insert(0, "/root/.axon_site")

import numpy as np
import ml_dtypes

import concourse.bass as bass
import concourse.mybir as mybir
import concourse.tile as tile
from concourse.vector_clock import ScopedClock
from concourse.bass_utils import run_bass_kernel_spmd

F32 = mybir.dt.float32
BF16 = mybir.dt.bfloat16
BF16_NP = ml_dtypes.bfloat16
FP8 = mybir.dt.float8e4
FP8_NP = mybir.dt.np(mybir.dt.float8e4)
W3_SCALE = 64.0
A2_SCALE = 4.0
UNSCALE = 1.0 / (W3_SCALE * A2_SCALE)
EXP = mybir.ActivationFunctionType.Exp
DR = mybir.MatmulPerfMode.DoubleRow

NCORES = 8
B, N, DIM, HEADS, DH = 16, 600, 128, 8, 32
INNER = HEADS * DH          # 256
D3 = 3 * DIM                # 384
QKV = 3 * DH                # 96 hypernet e-columns per head
W3_SL = DIM * QKV           # 12288 w3 cols per core (d-major x 96e)
V3_SL = 32 * DIM            # 4096 v3 cols per core (32 iv-rows)
SCALE = DH ** -0.5
# a2a payload (bf16 elems): [po|s-4x] block 72x300, r_v 32, pad, v3h 4096
PO_W = 72 * 300             # 21600
RV_OFF = PO_W               # 21600
V3_OFF = 21696
A2A_W = V3_OFF + V3_SL      # 25792
# m-tile column offsets inside the two big dots PSUM tiles and the
# in-bank matmul splits (psum bank = 512 f32 cols)
A_TILES = [(0, [(0, 512), (512, 88)]),
           (600, [(600, 424), (1024, 176)]),
           (1200, [(1200, 336), (1536, 264)])]
B_TILES = [(0, [(0, 512), (512, 88)]),
           (600, [(600, 424), (1024, 176)])]
MT_OF = [(0, 0), (0, 1), (0, 2), (1, 0), (1, 1)]  # m-tile -> (tile, idx)
BATCH_ORDER = list(range(16))


# ---------------------------------------------------------------------------
# This walrus build accepts at most ONE sync wait / update per instruction;
# Tile can emit more. Split extras onto adjacent same-engine NoOps.
class _SplitWaitTileContext(tile.TileContext):
    def _split_sync(self, insts):
        out = []
        for inst in insts:
            si = inst.sync_info
            if si is None:
                out.append(inst)
                continue
            waits = list(si.on_wait) if si.on_wait else []
            updates = list(si.on_update) if si.on_update else []
            if len(waits) <= 1 and len(updates) <= 1:
                out.append(inst)
                continue
            for w in waits[1:]:
                nop = mybir.InstNoOp(name=f"I-{self.nc.next_id()}", ins=[], outs=[])
                nop.engine = inst.engine
                nop.sync_info = mybir.SyncInfo(on_wait=[w], on_update=[])
                out.append(nop)
            inst.sync_info = mybir.SyncInfo(on_wait=waits[:1], on_update=updates[:1])
            out.append(inst)
            for u in updates[1:]:
                nop = mybir.InstNoOp(name=f"I-{self.nc.next_id()}", ins=[], outs=[])
                nop.engine = inst.engine
                nop.sync_info = mybir.SyncInfo(on_wait=[], on_update=[u])
                out.append(nop)
        return out

    def _lower_ordered_insts(self, ordered):
        for bb_name in list(ordered.keys()):
            ordered[bb_name] = self._split_sync(ordered[bb_name])
        return super()._lower_ordered_insts(ordered)

    def _drain_and_barrier(self, tick_clock, wait_clock):
        nc = self.nc
        probe = nc.sync.nop()
        wait_clock.add_sem_waits(probe.ins, ScopedClock({None: tick_clock.global_clock}))
        si = probe.ins.sync_info
        waits = list(si.on_wait) if si is not None and si.on_wait else []
        if len(waits) > 1:
            probe.ins.sync_info = mybir.SyncInfo(on_wait=waits[:1], on_update=[])
            for w in waits[1:]:
                extra = nc.sync.nop()
                extra.ins.sync_info = mybir.SyncInfo(on_wait=[w], on_update=[])
        nc.sync.drain()
        nc.all_engine_barrier()
        assert self.sems is not None
        popped = nc._tile_sem_poison_stack.pop()
        assert popped is self._sem_poison
        nc.clear_and_free_semaphores(list(self.sems.allocated().values()))
        nc.all_engine_barrier()


# ---------------------------------------------------------------------------
def _build_program(with_bias):
    b12, c12, b3f, c3f = with_bias
    nc = bass.Bass("TRN2", target_bir_lowering=False, debug=False,
                   num_devices=NCORES)

    xT = nc.dram_tensor("xT", [B, DIM, N], BF16, kind="ExternalInput")
    maskT = nc.dram_tensor("maskT", [B, N, N], BF16, kind="ExternalInput")
    frateT = nc.dram_tensor("frateT", [2, B], F32, kind="ExternalInput")
    w1 = nc.dram_tensor("w1", [2, D3], F32, kind="ExternalInput")
    w2 = nc.dram_tensor("w2", [D3, D3], F32, kind="ExternalInput")
    w3c = nc.dram_tensor("w3c", [D3, W3_SL], FP8, kind="ExternalInput")
    selm = nc.dram_tensor("selm", [32, INNER], BF16, kind="ExternalInput")
    v1 = nc.dram_tensor("v1", [2, INNER], F32, kind="ExternalInput")
    v2 = nc.dram_tensor("v2", [INNER, INNER], F32, kind="ExternalInput")
    v3c = nc.dram_tensor("v3c", [INNER, V3_SL], FP8, kind="ExternalInput")
    if b12:
        b1t = nc.dram_tensor("b1t", [D3, 1], F32, kind="ExternalInput")
        b2t = nc.dram_tensor("b2t", [D3, 1], F32, kind="ExternalInput")
    if c12:
        c1t = nc.dram_tensor("c1t", [INNER, 1], F32, kind="ExternalInput")
        c2t = nc.dram_tensor("c2t", [INNER, 1], F32, kind="ExternalInput")
    if b3f:
        b3c = nc.dram_tensor("b3c", [1, W3_SL], BF16, kind="ExternalInput")
    if c3f:
        c3c = nc.dram_tensor("c3c", [1, V3_SL], BF16, kind="ExternalInput")
    yT = nc.dram_tensor("yT", [2, DIM, N], F32, kind="ExternalOutput")

    with _SplitWaitTileContext(nc) as tc:
        with (
            tc.tile_pool(name="const", bufs=1) as cpool,
            tc.tile_pool(name="wts", bufs=1) as wpool,
            tc.tile_pool(name="achain", bufs=1) as apool,
            tc.tile_pool(name="hyp", bufs=3) as hpool,
            tc.tile_pool(name="xs", bufs=1) as xpool,
            tc.tile_pool(name="masks", bufs=5) as mpool,
            tc.tile_pool(name="small", bufs=3) as spool,
            tc.tile_pool(name="qk", bufs=2) as qpool,
            tc.tile_pool(name="ee", bufs=5) as epool,
            tc.tile_pool(name="emm", bufs=5) as empool,
            tc.tile_pool(name="stg", bufs=2) as gpool,
            tc.tile_pool(name="tail", bufs=1) as tpool,
            # PSUM: D = rotating 2-bank tiles (proj/dots/tail), P = po/s
            # accumulator bank, C = colsum bank
            tc.tile_pool(name="psD", bufs=3, space="PSUM") as psD,
            tc.tile_pool(name="psP", bufs=1, space="PSUM") as psP,
            tc.tile_pool(name="psC", bufs=1, space="PSUM") as psC,
            tc.tile_pool(name="dram", bufs=1, space="DRAM") as dpool,
        ):
            ones32 = cpool.tile([DIM, 32], BF16, name="ones32")
            nc.vector.memset(ones32[:], 1.0)
            ones_col = ones32[:, 0:1]

            # ---- small weights in
            fr_sb = wpool.tile([2, B], F32, name="fr_sb")
            nc.sync.dma_start(out=fr_sb[:], in_=frateT[:])
            w1_sb = wpool.tile([2, D3], F32, name="w1_sb")
            nc.sync.dma_start(out=w1_sb[:], in_=w1[:])
            w2_sb = [wpool.tile([DIM, D3], F32, name=f"w2_sb{k}") for k in range(3)]
            for k in range(3):
                nc.sync.dma_start(out=w2_sb[k][:], in_=w2[128 * k:128 * (k + 1), :])
            v1_sb = wpool.tile([2, INNER], F32, name="v1_sb")
            nc.sync.dma_start(out=v1_sb[:], in_=v1[:])
            v2_sb = [wpool.tile([DIM, INNER], F32, name=f"v2_sb{k}") for k in range(2)]
            for k in range(2):
                nc.sync.dma_start(out=v2_sb[k][:], in_=v2[128 * k:128 * (k + 1), :])
            sel_sb = wpool.tile([32, INNER], BF16, name="sel_sb")
            nc.sync.dma_start(out=sel_sb[:], in_=selm[:])
            bias_sb = {}
            if b12:
                bias_sb["b1"] = wpool.tile([D3, 1], F32, name="b1_sb")
                nc.sync.dma_start(out=bias_sb["b1"][:], in_=b1t[:])
                bias_sb["b2"] = wpool.tile([D3, 1], F32, name="b2_sb")
                nc.sync.dma_start(out=bias_sb["b2"][:], in_=b2t[:])
            if c12:
                bias_sb["c1"] = wpool.tile([INNER, 1], F32, name="c1_sb")
                nc.sync.dma_start(out=bias_sb["c1"][:], in_=c1t[:])
                bias_sb["c2"] = wpool.tile([INNER, 1], F32, name="c2_sb")
                nc.sync.dma_start(out=bias_sb["c2"][:], in_=c2t[:])
            if b3f:
                b3_sb = wpool.tile([1, W3_SL], BF16, name="b3_sb")
                nc.sync.dma_start(out=b3_sb[:], in_=b3c[:])
                ones_row16 = cpool.tile([1, B], BF16, name="ones_row16")
                nc.vector.memset(ones_row16[:], 1.0)
            if c3f:
                c3_sb = wpool.tile([1, V3_SL], BF16, name="c3_sb")
                nc.sync.dma_start(out=c3_sb[:], in_=c3c[:])
                if not b3f:
                    ones_row16 = cpool.tile([1, B], BF16, name="ones_row16")
                    nc.vector.memset(ones_row16[:], 1.0)

            # ---- big hypernet weight slices (scoped: freed after use)
            w3ctx = tc.tile_pool(name="w3", bufs=1)
            w3pool = w3ctx.__enter__()
            W3Q = W3_SL // 4
            w3view = w3c[:].rearrange("(ks p) n -> p ks n", p=DIM)
            w3dr4 = []
            for q4 in range(4):
                q0 = q4 * W3Q
                t = w3pool.tile([DIM, 3, W3Q], FP8, name=f"w3dr{q4}")
                nc.sync.dma_start(out=t[:], in_=w3view[:, :, q0:q0 + W3Q])
                w3dr4.append(t)
            v3dr = w3pool.tile([DIM, 2, V3_SL], FP8, name="v3dr")
            nc.scalar.dma_start(
                out=v3dr[:], in_=v3c[:].rearrange("(ks p) n -> p ks n", p=DIM))

            xT_sb = [xpool.tile([DIM, N], BF16, name=f"xT_sb{b}")
                     for b in range(B)]
            mask_sb = {}

            def fetch_mask(b, eng=None):
                # one DMA: [600, 600] -> [128, 5, 600] (m-tile-major view)
                eng = eng or nc.sync
                t = mpool.tile([DIM, 5, N], BF16, name=f"mask{b}", tag="mask")
                eng.dma_start(
                    out=t[:, 0:4, :],
                    in_=maskT[b, 0:512, :].rearrange("(mt p) n -> p mt n",
                                                     p=DIM))
                eng.dma_start(out=t[:88, 4, :], in_=maskT[b, 512:600, :])
                mask_sb[b] = [t[:, mt, :] for mt in range(5)]



            # ---- a-chain: a1T = (frate@W1+b1).T as 3x[128,16]
            a1T = []
            for t in range(3):
                p = psD.tile([DIM, 1024], F32, name="pa", tag="psD")
                nc.tensor.matmul(p[:, :B], w1_sb[:, 128 * t:128 * (t + 1)],
                                 fr_sb[:], start=True, stop=True)
                s = apool.tile([DIM, B], F32, name=f"a1T{t}")
                if b12:
                    nc.scalar.activation(s[:], p[:, :B],
                                         mybir.ActivationFunctionType.Identity,
                                         bias=bias_sb["b1"][128 * t:128 * (t + 1), :])
                else:
                    nc.scalar.copy(s[:], p[:, :B])
                a1T.append(s)
            a2f8 = apool.tile([DIM, 3, B], FP8, name="a2f8")
            for t in range(3):
                p = psD.tile([DIM, 1024], F32, name="pa2", tag="psD")
                for k in range(3):
                    nc.tensor.matmul(p[:, :B], w2_sb[k][:, 128 * t:128 * (t + 1)],
                                     a1T[k][:], start=(k == 0), stop=(k == 2))
                if b12:
                    tmp = apool.tile([DIM, B], F32, name=f"a2tmp{t}")
                    nc.scalar.activation(tmp[:], p[:, :B],
                                         mybir.ActivationFunctionType.Identity,
                                         bias=bias_sb["b2"][128 * t:128 * (t + 1), :])
                    with nc.allow_low_precision("fp8 hypernet activations"):
                        nc.vector.tensor_scalar_mul(a2f8[:, t, :], tmp[:],
                                                    A2_SCALE)
                else:
                    nc.scalar.mul(a2f8[:, t, :], p[:, :B], A2_SCALE)
            av1T = []
            for t in range(2):
                p = psD.tile([DIM, 1024], F32, name="pav", tag="psD")
                nc.tensor.matmul(p[:, :B], v1_sb[:, 128 * t:128 * (t + 1)],
                                 fr_sb[:], start=True, stop=True)
                s = apool.tile([DIM, B], F32, name=f"av1T{t}")
                if c12:
                    nc.scalar.activation(s[:], p[:, :B],
                                         mybir.ActivationFunctionType.Identity,
                                         bias=bias_sb["c1"][128 * t:128 * (t + 1), :])
                else:
                    nc.scalar.copy(s[:], p[:, :B])
                av1T.append(s)
            avf8 = apool.tile([DIM, 2, B], FP8, name="avf8")
            for t in range(2):
                p = psD.tile([DIM, 1024], F32, name="pav2", tag="psD")
                for k in range(2):
                    nc.tensor.matmul(p[:, :B], v2_sb[k][:, 128 * t:128 * (t + 1)],
                                     av1T[k][:], start=(k == 0), stop=(k == 1))
                if c12:
                    tmp = apool.tile([DIM, B], F32, name=f"avtmp{t}")
                    nc.scalar.activation(tmp[:], p[:, :B],
                                         mybir.ActivationFunctionType.Identity,
                                         bias=bias_sb["c2"][128 * t:128 * (t + 1), :])
                    with nc.allow_low_precision("fp8 hypernet activations"):
                        nc.vector.tensor_scalar_mul(avf8[:, t, :], tmp[:],
                                                    A2_SCALE)
                else:
                    nc.scalar.mul(avf8[:, t, :], p[:, :B], A2_SCALE)

            # ---- W3 phase: h*256 for all 16 batches -> hbuf (DRAM roundtrip)
            # (values are h*W3_SCALE*A2_SCALE; the exp later applies UNSCALE)
            a2a_in = dpool.tile([B, A2A_W], BF16, name="a2a_in")
            a2a_out = dpool.tile([B, A2A_W], BF16, name="a2a_out")
            hbuf = dpool.tile([B, W3_SL], BF16, name="hbuf")
            CH = 512
            for j in range(W3_SL // CH):
                p = psD.tile([B, 1024], F32, name="ph", tag="psD")
                w3t = w3dr4[j // 6]
                jj = j % 6
                nc.tensor.matmul(p[:, 0:CH], a2f8[:, 0:2, :],
                                 w3t[:, 0:2, CH * jj:CH * (jj + 1)],
                                 start=True, stop=False, perf_mode=DR)
                nc.tensor.matmul(p[:, 0:CH], a2f8[:, 2, :],
                                 w3t[:, 2, CH * jj:CH * (jj + 1)],
                                 start=False, stop=not b3f)
                if b3f:
                    nc.tensor.matmul(p[:, 0:CH], ones_row16[:],
                                     b3_sb[:, CH * j:CH * (j + 1)],
                                     start=False, stop=True)
                hs = hpool.tile([B, CH], BF16, name="hs", tag="hs")
                if j % 2 == 0:
                    nc.scalar.copy(hs[:], p[:, 0:CH])
                else:
                    nc.vector.tensor_copy(hs[:], p[:, 0:CH])
                nc.sync.dma_start(out=hbuf[:, CH * j:CH * (j + 1)], in_=hs[:])
            for j in range(V3_SL // CH):
                p = psD.tile([B, 1024], F32, name="phv", tag="psD")
                nc.tensor.matmul(p[:, 0:CH], avf8[:, 0:2, :],
                                 v3dr[:, 0:2, CH * j:CH * (j + 1)],
                                 start=True, stop=not c3f, perf_mode=DR)
                if c3f:
                    nc.tensor.matmul(p[:, 0:CH], ones_row16[:],
                                     c3_sb[:, CH * j:CH * (j + 1)],
                                     start=False, stop=True)
                hv = hpool.tile([B, CH], BF16, name="hv", tag="hs")
                if j % 2 == 0:
                    nc.scalar.copy(hv[:], p[:, 0:CH])
                else:
                    nc.vector.tensor_copy(hv[:], p[:, 0:CH])
                # ship v3h chunk rows straight into the a2a payload
                nc.sync.dma_start(
                    out=a2a_in[:, V3_OFF + CH * j:V3_OFF + CH * (j + 1)],
                    in_=hv[:])

            w3ctx.__exit__(None, None, None)

            # ---- x and first masks now (after hypernet weights had the
            # DMA engines to themselves)
            for b in range(B):
                nc.scalar.dma_start(out=xT_sb[b][:], in_=xT[b])
            for k in range(2):
                fetch_mask(BATCH_ORDER[k])

            hview = hbuf[:].rearrange("b (d e) -> b d e", d=DIM)

            # ================= batch-owner: finish my 2 batches ============
            # heads regrouped 3|3|2 so every PE operand base is 0/32/64
            H_T = [(0, 3), (3, 3), (6, 2)]

            def owner_tail(k):
                src_ = a2a_out[:].rearrange("(s two) w -> two s w", two=2)[k]
                poview = a2a_out[:, 0:PO_W].rearrange(
                    "(s two) (p n) -> two s p n", two=2, p=72)[k]
                outT = [tpool.tile([32 * cnt, N], BF16, name=f"outT{k}_{t}",
                                   tag=f"outT{t}")
                        for t, (h0, cnt) in enumerate(H_T)]
                s32 = tpool.tile([32, N], BF16, name=f"s32_{k}", tag="s32")
                for t, (h0, cnt) in enumerate(H_T):
                    for hf in range(2):
                        eng = nc.sync if hf == 0 else nc.scalar
                        eng.dma_start(
                            out=outT[t][:, 300 * hf:300 * hf + 300],
                            in_=poview[h0:h0 + cnt, 36 * hf:36 * hf + 32, :])
                for hf in range(2):
                    nc.sync.dma_start(
                        out=s32[:, 300 * hf:300 * hf + 300],
                        in_=poview[:, 36 * hf + 32:36 * hf + 36, :])
                # 1/s (dense, split halves to shorten the chain), scatter
                s32f = tpool.tile([32, N], F32, name=f"s32f_{k}", tag="s32f")
                rs32f = tpool.tile([32, N], F32, name=f"rs32f_{k}",
                                   tag="rs32f")
                rs32 = tpool.tile([32, N], BF16, name=f"rs32_{k}", tag="rs32")
                for hf in range(2):
                    h = slice(300 * hf, 300 * hf + 300)
                    nc.vector.tensor_copy(s32f[:, h], s32[:, h])
                    nc.vector.reciprocal(rs32f[:, h], s32f[:, h])
                    with nc.allow_low_precision("normalizer bf16"):
                        nc.vector.tensor_copy(rs32[:, h], rs32f[:, h])
                rvcol = [tpool.tile([32 * cnt, 1], BF16, name=f"rvc{k}_{t}",
                                    tag=f"rvc{t}")
                         for t, (h0, cnt) in enumerate(H_T)]
                ehw = []
                for t, (h0, cnt) in enumerate(H_T):
                    nc.scalar.dma_start(
                        out=rvcol[t][:],
                        in_=src_[h0:h0 + cnt, RV_OFF:RV_OFF + DH])
                    raw = tpool.tile([32 * cnt, DIM], BF16,
                                     name=f"ehwr{k}_{t}", tag=f"ehwr{t}")
                    nc.scalar.dma_start(
                        out=raw[:],
                        in_=src_[h0:h0 + cnt, V3_OFF:V3_OFF + V3_SL]
                        .rearrange("s (i d) -> s i d", d=DIM))
                    e = tpool.tile([32 * cnt, DIM], BF16, name=f"ehw{k}_{t}",
                                   tag=f"ehw{t}")
                    nc.scalar.activation(e[:], raw[:], EXP, scale=UNSCALE)
                    ehw.append(e)
                # ow colsum over iv (raw ehw) -> per-partition recip
                psw = psP.tile([DIM, 512], F32, name="psw", tag="psP")
                for t, (h0, cnt) in enumerate(H_T):
                    nc.tensor.matmul(psw[:, 0:1], ehw[t][:],
                                     ones32[:32 * cnt, 0:1],
                                     start=(t == 0), stop=(t == 2))
                row_ = tpool.tile([DIM, 1], F32, name=f"row{k}", tag="row")
                nc.vector.reciprocal(row_[:], psw[:, 0:1])
                # R[e, n] = rs[head(e), n] outer-built; rv folds into ehw
                pY = psD.tile([DIM, 1024], F32, name="pY", tag="psD")
                outn = []
                ehwn = []
                for t, (h0, cnt) in enumerate(H_T):
                    pR = psD.tile([DIM, 1024], F32, name=f"pR{t}", tag="psD")
                    for n0, nw in ((0, 512), (512, 88)):
                        nc.tensor.matmul(
                            pR[0:32 * cnt, n0:n0 + nw],
                            sel_sb[:, 32 * h0:32 * (h0 + cnt)],
                            rs32[:, n0:n0 + nw], start=True, stop=True)
                    on = tpool.tile([32 * cnt, N], BF16, name=f"outn{k}_{t}",
                                    tag=f"outn{t}")
                    nc.vector.tensor_mul(on[:], outT[t][:],
                                         pR[0:32 * cnt, 0:N])
                    outn.append(on)
                    rvf = tpool.tile([32 * cnt, 1], F32,
                                     name=f"rvf{k}_{t}", tag=f"rvf{t}")
                    nc.vector.tensor_copy(rvf[:], rvcol[t][:])
                    en = tpool.tile([32 * cnt, DIM], BF16,
                                    name=f"ehwn{k}_{t}", tag=f"ehwn{t}")
                    nc.vector.tensor_scalar_mul(en[:], ehw[t][:], rvf[:])
                    ehwn.append(en)
                for n0, nw in ((0, 512), (512, 88)):
                    for t in range(3):
                        nc.tensor.matmul(pY[:, n0:n0 + nw], ehwn[t][:],
                                         outn[t][:, n0:n0 + nw],
                                         start=(t == 0), stop=(t == 2))
                ys = tpool.tile([DIM, N], F32, name=f"ys{k}", tag="ys")
                nc.vector.tensor_scalar_mul(ys[:], pY[:, 0:N], row_[:])
                nc.sync.dma_start(out=yT[k], in_=ys[:])


            # ================= attention: my head, all 16 batches ==========
            # The PE stream is kept dense: rowsum/attn@v groups and the
            # po/s staging are deferred closures flushed ~4 PE-slots behind
            # the dots that feed them, across batch boundaries.
            pending = []
            pending_pP = [None]

            def flush(n):
                for _ in range(n):
                    if pending:
                        pending.pop(0)()

            for it, b in enumerate(BATCH_ORDER):
                if it + 2 < B:
                    fetch_mask(BATCH_ORDER[it + 2])

                qkvraw = spool.tile([DIM, QKV], BF16, name="qkvraw",
                                    tag="qkvraw")
                nc.sync.dma_start(out=qkvraw[:], in_=hview[b])
                ehq = spool.tile([DIM, QKV], BF16, name="ehq", tag="ehq")
                nc.scalar.activation(ehq[:], qkvraw[:], EXP, scale=UNSCALE)
                pC = psC.tile([DH, 512], F32, name="pC", tag="psC")
                for j in range(3):
                    nc.tensor.matmul(pC[:, j:j + 1],
                                     ehq[:, DH * j:DH * (j + 1)], ones_col[:],
                                     start=True, stop=True)
                flush(1)  # s2-m3 of previous batch
                r3 = spool.tile([DH, 3], F32, name="r3", tag="r3")
                nc.vector.reciprocal(r3[:], pC[:, 0:3])
                rc = spool.tile([DH, 1], F32, name="rc", tag="rc")
                nc.vector.tensor_mul(rc[:], r3[:, 0:1], r3[:, 1:2])
                nc.vector.tensor_scalar_mul(rc[:], rc[:], SCALE)
                rvb = spool.tile([DH, 1], BF16, name="rvb", tag="rvb")
                with nc.allow_low_precision("ship r_v bf16"):
                    nc.vector.tensor_copy(rvb[:], r3[:, 2:3])
                nc.sync.dma_start(
                    out=a2a_in[b, RV_OFF:RV_OFF + DH], in_=rvb[:])

                # proj q, k (two psD tiles), v (third)
                pq = psD.tile([DIM, 1024], F32, name="pq", tag="psD")
                pk = psD.tile([DIM, 1024], F32, name="pk", tag="psD")
                pv = psD.tile([DIM, 1024], F32, name="pv", tag="psD")
                for pt, lo in ((pq, 0), (pk, DH)):
                    for n0, nw in ((0, 512), (512, 88)):
                        nc.tensor.matmul(pt[0:DH, n0:n0 + nw],
                                         ehq[:, lo:lo + DH],
                                         xT_sb[b][:, n0:n0 + nw],
                                         start=True, stop=True)
                flush(1)  # s2-m4 of previous batch
                for mt in range(5):
                    msz = 128 if mt < 4 else 88
                    nc.tensor.matmul(
                        pv[:msz, 32 * mt:32 * mt + 32],
                        xT_sb[b][:, 128 * mt:128 * mt + msz],
                        ehq[:, 2 * DH:QKV], start=True, stop=True)
                qkT = qpool.tile([DH, 1200], BF16, name="qkT", tag="qkT")
                nc.scalar.mul(qkT[:, 0:N], pq[0:DH, 0:N], rc[:])
                with nc.allow_low_precision("kT bf16 copy"):
                    nc.vector.tensor_copy(qkT[:, N:2 * N], pk[0:DH, 0:N])
                v_sb = spool.tile([DIM, 160], BF16, name="v_sb", tag="v_sb")
                nc.vector.tensor_copy(v_sb[:], pv[:, 0:160])
                flush(1)  # ship of previous batch

                for mt in range(5):
                    msz = 128 if mt < 4 else 88
                    pd = psD.tile([DIM, 1024], F32, name="pd", tag="psD")
                    for n0, nw in ((0, 512), (512, 88)):
                        nc.tensor.matmul(
                            pd[:msz, n0:n0 + nw],
                            qkT[:, N + 128 * mt:N + 128 * mt + msz],
                            qkT[:, n0:n0 + nw], start=True, stop=True)
                    e_t = epool.tile([DIM, N], BF16, name="e_t", tag="e")
                    nc.scalar.activation(e_t[:msz, :], pd[:msz, 0:N], EXP)
                    em_t = empool.tile([DIM, N], BF16, name="em_t", tag="em")

                    def s2(mt=mt, msz=msz, e_t=e_t, em_t=em_t, b=b,
                           v_sb=v_sb):
                        eng = nc.vector if mt in (0, 2) else nc.gpsimd
                        eng.tensor_mul(em_t[:msz, :], e_t[:msz, :],
                                       mask_sb[b][mt][:msz, :])
                        if mt == 0:
                            pending_pP[0] = psP.tile([DIM, 512], F32,
                                                     name="pP", tag="psP")
                        pP = pending_pP[0]
                        for hf in range(2):
                            n0 = 300 * hf
                            kw = ({} if hf == 0
                                  else {"tile_position": (0, 96)})
                            nc.tensor.matmul(
                                pP[32 + 64 * hf:36 + 64 * hf, 0:300],
                                ones32[:msz, 0:4],
                                e_t[:msz, n0:n0 + 300],
                                start=(mt == 0), stop=(mt == 4), **kw)
                            nc.tensor.matmul(
                                pP[64 * hf:64 * hf + 32, 0:300],
                                v_sb[:msz, 32 * mt:32 * mt + 32],
                                em_t[:msz, n0:n0 + 300],
                                start=(mt == 0), stop=(mt == 4))
                    pending.append(s2)
                    if mt == 3:
                        flush(1)
                    elif mt == 4:
                        flush(2)

                def ship(b=b):
                    pP = pending_pP[0]
                    postile = gpool.tile([100, 300], BF16, name="postile",
                                         tag="postile")
                    with nc.allow_low_precision("ship po/s bf16"):
                        nc.vector.tensor_copy(postile[:], pP[0:100, 0:300])
                    nc.sync.dma_start(out=a2a_in[b, 0:36 * 300],
                                      in_=postile[0:36, :])
                    nc.sync.dma_start(out=a2a_in[b, 36 * 300:PO_W],
                                      in_=postile[64:100, :])
                pending.append(ship)
            flush(len(pending))
            nc.gpsimd.collective_compute(
                "AllToAll", mybir.AluOpType.bypass,
                replica_groups=[list(range(NCORES))],
                ins=[a2a_in[:]], outs=[a2a_out[:]],
            )
            owner_tail(0)

            owner_tail(1)

    return nc


_PROGRAM_CACHE = {}


def _get_program(with_bias):
    if with_bias not in _PROGRAM_CACHE:
        _PROGRAM_CACHE[with_bias] = _build_program(with_bias)
    return _PROGRAM_CACHE[with_bias]


def _shard_inputs(x, mask, resolution, framerate,
                  W1, b1, W2, b2, W3, b3, V1, c1, V2, c2, V3, c3, with_bias):
    b12, c12, b3f, c3f = with_bias
    x = np.asarray(x, np.float32)
    mask = np.asarray(mask, np.float32)
    xT = np.ascontiguousarray(x.transpose(0, 2, 1)).astype(BF16_NP)
    maskT = np.ascontiguousarray(
        mask[0, :, 0].transpose(0, 2, 1)).astype(BF16_NP)
    frateT = np.ascontiguousarray(
        np.stack([np.asarray(framerate, np.float32),
                  np.asarray(resolution, np.float32)], axis=0))
    W1 = np.ascontiguousarray(np.asarray(W1, np.float32))
    W2 = np.ascontiguousarray(np.asarray(W2, np.float32))
    V1 = np.ascontiguousarray(np.asarray(V1, np.float32))
    V2 = np.ascontiguousarray(np.asarray(V2, np.float32))
    W3v = np.asarray(W3, np.float32).reshape(D3, DIM, 3 * INNER)
    V3v = np.asarray(V3, np.float32).reshape(INNER, INNER, DIM)
    if b3f:
        b3v = np.asarray(b3, np.float32).reshape(DIM, 3 * INNER)
    if c3f:
        c3v = np.asarray(c3, np.float32).reshape(INNER, DIM)
    SELM = np.zeros((32, INNER), np.float32)
    for e in range(INNER):
        SELM[4 * (e // 32), e] = 1.0
    SELM = SELM.astype(BF16_NP)
    in_maps = []
    for c in range(NCORES):
        # head-c e-columns of W3: q | k | v blocks, d-major
        ecols = np.r_[32 * c:32 * c + 32, INNER + 32 * c:INNER + 32 * c + 32,
                      2 * INNER + 32 * c:2 * INNER + 32 * c + 32]
        w3slice = W3v[:, :, ecols].reshape(D3, W3_SL)
        m = {
            "xT": xT,
            "maskT": maskT,
            "frateT": frateT,
            "w1": W1, "w2": W2, "v1": V1, "v2": V2,
            "w3c": (w3slice * W3_SCALE).astype(FP8_NP),
            "selm": SELM,
            "v3c": (np.ascontiguousarray(
                V3v[:, 32 * c:32 * (c + 1), :]).reshape(INNER, V3_SL)
                * W3_SCALE).astype(FP8_NP),
        }
        if b12:
            m["b1t"] = np.asarray(b1, np.float32).reshape(D3, 1)
            m["b2t"] = np.asarray(b2, np.float32).reshape(D3, 1)
        if c12:
            m["c1t"] = np.asarray(c1, np.float32).reshape(INNER, 1)
            m["c2t"] = np.asarray(c2, np.float32).reshape(INNER, 1)
        if b3f:
            m["b3c"] = (b3v[:, ecols].reshape(1, W3_SL)
                        * (W3_SCALE * A2_SCALE)).astype(BF16_NP)
        if c3f:
            m["c3c"] = (np.ascontiguousarray(
                c3v[32 * c:32 * (c + 1), :].reshape(1, V3_SL))
                * (W3_SCALE * A2_SCALE)).astype(BF16_NP)
        in_maps.append(m)
    return in_maps


def _run(inputs, trace=False, tmpdir=None):
    with_bias = (
        bool(np.any(inputs["b1"])) or bool(np.any(inputs["b2"])),
        bool(np.any(inputs["c1"])) or bool(np.any(inputs["c2"])),
        bool(np.any(inputs["b3"])),
        bool(np.any(inputs["c3"])),
    )
    nc = _get_program(with_bias)
    in_maps = _shard_inputs(with_bias=with_bias, **inputs)
    res = run_bass_kernel_spmd(nc, in_maps, core_ids=list(range(NCORES)),
                               trace=trace, tmpdir=tmpdir)
    outs = []
    for c in range(NCORES):
        yt = res.results[c]["yT"]  # [2, 128, 600] = batches 2c, 2c+1
        outs.append(yt.transpose(0, 2, 1))
    full = np.ascontiguousarray(np.concatenate(outs, axis=0)).astype(np.float32)
    return full, res


def kernel(**inputs) -> np.ndarray:
    out, _ = _run(inputs, trace=False)
    return out
